# revision 1
# baseline (speedup 1.0000x reference)
"""Bilateral blur (7x7, L1 color distance) on 8 Trainium2 NeuronCores.

Input (4, 3, 512, 512) fp32 -> output (4, 3, 512, 512) fp32.

Sharding: pure data parallelism - core i handles batch i//2, row-half i%2
(256 output rows x 512 cols). The host reflect-pads each image and re-lays
each shard into a "transposed-blocked" layout: partition p (0..127) owns
output columns [4p, 4p+4); its free dim holds, per channel, a 10x262 patch
(padded cols 4p..4p+9 x 262 padded rows, flattened xl*262+y). TRN2 compute
engines cannot read across partitions, so this layout turns all 49 tap
shifts (dy, dx) into pure free-dim AP offsets (dx*262 + dy).

Per tap: dlt = taps-center (fat fp16 TT over 3 channels), |dlt| (ACT Abs),
d = ch-sum (2 TT), q = d^2 (ACT Square), w = exp(-50q + ln s_k) (ACT Exp,
scale/bias immediates), products w*tap (fp16 TT), and a single fat fp32
accumulate of [m0|m1|m2|w]. Final out = num * reciprocal(den).

fp16 notes: all hot DVE ops are TensorTensor (2x DVE mode in
fp16; the TensorScalarPtr family is 1x-only on this ISA so it is avoided).
-50 rides the exp's scale immediate; ln(s_k) rides its per-partition bias AP
(from a small DMA'd table). Taps are read from one of two host-provided fp16
copies (second shifted one row) so every tap AP is 4-byte aligned (dy parity),
which the DVE 2x mode requires.
"""
import numpy as np

import concourse.bass as bass
import concourse.bacc as bacc
import concourse.mybir as mybir
from concourse.tile import TileContext
from concourse import bass_utils

C = 3
B, H, W = 4, 512, 512
KX = KY = 7
PAD = 3
SIGMA_COLOR = 0.1
N_CORES = 8

ROWS = 256
WG = 4
NPART = 128
XE, YE = WG + 2 * PAD, ROWS + 2 * PAD
FREE_IN = XE * YE
FREE_OUT = WG * ROWS
NTAP = KY * KX


def _space_kernel():
    def g1(k, sigma):
        x = np.arange(k, dtype=np.float64) - (k - 1) / 2.0
        g = np.exp(-0.5 * (x / sigma) ** 2)
        return g / g.sum()
    gy, gx = g1(KY, 1.5), g1(KX, 1.5)
    return (gy[:, None] * gx[None, :]).reshape(-1)


def _tap_ap(t, dx, dy, nch=C, ch0=0, dy_base=0):
    a = t[:]
    return bass.AP(a.tensor, a.offset + ch0 * FREE_IN + dx * YE + (dy - dy_base),
                   [[C * FREE_IN, NPART], [FREE_IN, nch], [YE, WG], [1, ROWS]])


def _stk_ap(t, nch=C, ch0=0, step0=False):
    a = t[:]
    tot = a.shape[1]
    return bass.AP(a.tensor, a.offset + ch0 * FREE_OUT,
                   [[tot, NPART], [0 if step0 else FREE_OUT, nch], [ROWS, WG], [1, ROWS]])


def _build(ntaps=NTAP):
    nc = bacc.Bacc()
    f32 = mybir.dt.float32
    f16 = mybir.dt.float16
    xe = nc.dram_tensor("xe", [NPART, C * FREE_IN], f16, kind="ExternalInput")
    xo = nc.dram_tensor("xo", [NPART, C * FREE_IN], f16, kind="ExternalInput")
    lnsb = nc.dram_tensor("lnsb", [NPART, NTAP], f32, kind="ExternalInput")
    ident = nc.dram_tensor("ident", [NPART, NPART], f16, kind="ExternalInput")
    o = nc.dram_tensor("o", [NPART, C * FREE_OUT], f32, kind="ExternalOutput")
    AOT = mybir.AluOpType
    F = FREE_OUT
    SCL = -0.5 / SIGMA_COLOR ** 2

    with TileContext(nc) as tc:
        with tc.tile_pool(name="persist", bufs=1) as pool, \
             tc.tile_pool(name="tmp", bufs=6) as tp, \
             tc.tile_pool(name="ps", bufs=1, space="PSUM") as psp:
            Te = pool.tile([NPART, C * FREE_IN], f16, name="Te")
            nc.sync.dma_start(Te[:], xe[:])
            To = pool.tile([NPART, C * FREE_IN], f16, name="To")
            nc.sync.dma_start(To[:], xo[:])
            bias = pool.tile([NPART, NTAP], f32, name="bias")
            nc.sync.dma_start(bias[:], lnsb[:])
            idt = pool.tile([NPART, NPART], f16, name="idt")
            nc.sync.dma_start(idt[:], ident[:])

            def tile_for(dy):
                return (Te, 0) if dy % 2 == 0 else (To, 1)

            # fp32 accumulator [m0|m1|m2|w] lives in PSUM: per tap, identity
            # matmuls stream mAll through the PE array, whose PSUM writes
            # accumulate natively in fp32 (exact for fp16 inputs). This takes
            # the entire fp32 accumulation off the DVE critical path.
            acc = psp.tile([NPART, 4 * F], f32, name="acc")
            qb = pool.tile([NPART, F], f16, name="qb")
            MN = 512                       # matmul free-dim per PSUM bank

            tc_t, tc_b = tile_for(PAD)
            ctr = _tap_ap(tc_t, PAD, PAD, dy_base=tc_b)
            for dy in range(KY):
                for dx in range(KX):
                    k = dy * KX + dx
                    if k >= ntaps:
                        continue
                    tt, tb = tile_for(dy)
                    dlt = tp.tile([NPART, C * F], f16, name="dlt", tag="dlt")
                    nc.vector.tensor_tensor(out=_stk_ap(dlt),
                                            in0=_tap_ap(tt, dx, dy, dy_base=tb),
                                            in1=ctr, op=AOT.subtract)
                    adl = tp.tile([NPART, C * F], f16, name="adl", tag="adl")
                    # |.| split across engines: channels 0-1 on ACT, channel 2
                    # on DVE as a sign-bit mask over packed fp16 (u32 view)
                    nc.scalar.activation(adl[:, 0:2 * F], dlt[:, 0:2 * F],
                                         mybir.ActivationFunctionType.Abs,
                                         bias=0.0, scale=1.0)
                    nc.vector.tensor_scalar(
                        out=adl[:, 2 * F:].bitcast(mybir.dt.uint32),
                        in0=dlt[:, 2 * F:].bitcast(mybir.dt.uint32),
                        scalar1=0x7FFF7FFF, scalar2=None,
                        op0=AOT.bitwise_and)
                    # channel-sum on GPSIMD: takes ~1.3K cyc/tap off the DVE
                    # critical path; GPSIMD has slack and its SBUF-port draw
                    # is ~1% of the shared port's bandwidth
                    dsum = tp.tile([NPART, F], f16, name="dsum", tag="dsum")
                    nc.gpsimd.tensor_tensor(out=dsum[:], in0=adl[:, 0:F],
                                            in1=adl[:, F:2 * F], op=AOT.add)
                    nc.gpsimd.tensor_tensor(out=dsum[:], in0=dsum[:],
                                            in1=adl[:, 2 * F:], op=AOT.add)
                    # q = d^2 on ACT (same table set as Exp/Abs -> no set switch);
                    # keeps the DVE critical path shorter
                    nc.scalar.activation(qb[:], dsum[:],
                                         mybir.ActivationFunctionType.Square,
                                         bias=0.0, scale=1.0)
                    mAll = tp.tile([NPART, 4 * F], f16, name="mAll", tag="mAll")
                    # w = exp(-50*q + ln s_k) lands in the 4th slot
                    nc.scalar.activation(mAll[:, 3 * F:], qb[:],
                                         mybir.ActivationFunctionType.Exp,
                                         bias=bias[:, k:k + 1], scale=SCL)
                    wv = mAll[:, 3 * F:]
                    w_b3 = bass.AP(wv.tensor, wv.offset,
                                   [[4 * F, NPART], [0, 3], [ROWS, WG], [1, ROWS]])
                    nc.vector.tensor_tensor(out=_stk_ap(mAll, nch=3), in0=w_b3,
                                            in1=_tap_ap(tt, dx, dy, nch=3, dy_base=tb),
                                            op=AOT.mult)
                    # accumulate on the PE: 8 identity matmuls (one per bank)
                    for g in range(4 * F // MN):
                        nc.tensor.matmul(acc[:, g * MN:(g + 1) * MN], idt[:],
                                         mAll[:, g * MN:(g + 1) * MN],
                                         start=(k == 0), stop=(k == ntaps - 1))

            accm = pool.tile([NPART, 4 * F], f32, name="accm")
            nc.vector.tensor_copy(accm[:], acc[:])
            recip = pool.tile([NPART, F], f32, name="recip")
            nc.vector.reciprocal(recip[:], accm[:, 3 * F:])
            ot = pool.tile([NPART, C * F], f32, name="ot")
            nc.vector.tensor_tensor(out=_stk_ap(ot), in0=accm[:, 0:3 * F],
                                    in1=_stk_ap(recip, step0=True), op=AOT.mult)
            nc.sync.dma_start(o[:], ot[:])
    return nc


_COLIDX = np.arange(NPART)[:, None] * WG + np.arange(XE)[None, :]


def _shard_layout(shard, yshift):
    buf = np.zeros((NPART, C, XE, YE), np.float16)
    for c in range(C):
        blk = shard[c].T[_COLIDX]
        if yshift:
            buf[:, c, :, :YE - yshift] = blk[:, :, yshift:]
        else:
            buf[:, c] = blk
    return buf.reshape(NPART, C * FREE_IN)


_LNSB = np.broadcast_to(
    np.log(_space_kernel()).astype(np.float32)[None, :], (NPART, NTAP)).copy()

_NC_CACHE = {}


def _get_nc():
    if "nc" not in _NC_CACHE:
        nc = _build()
        nc.finalize()
        _NC_CACHE["nc"] = nc
    return _NC_CACHE["nc"]


def make_in_maps(x):
    xp = np.pad(x, ((0, 0), (0, 0), (PAD, PAD), (PAD, PAD)), mode="reflect")
    in_maps = []
    for core in range(N_CORES):
        b, half = core // 2, core % 2
        r0 = half * ROWS
        shard = xp[b, :, r0:r0 + ROWS + 2 * PAD, :]
        in_maps.append({"xe": _shard_layout(shard, 0),
                        "xo": _shard_layout(shard, 1),
                        "lnsb": _LNSB,
                        "ident": np.eye(NPART, dtype=np.float16)})
    return in_maps


def kernel(input: np.ndarray) -> np.ndarray:
    x = np.asarray(input, dtype=np.float32)
    assert x.shape == (B, C, H, W)
    in_maps = make_in_maps(x)
    nc = _get_nc()
    res = bass_utils.run_bass_kernel_spmd(nc, in_maps, list(range(N_CORES)))
    out = np.empty((B, C, H, W), np.float32)
    for core in range(N_CORES):
        b, half = core // 2, core % 2
        r0 = half * ROWS
        ov = np.asarray(res.results[core]["o"]).reshape(NPART, C, WG, ROWS)
        for c in range(C):
            out[b, c, r0:r0 + ROWS, :] = ov[:, c].transpose(2, 0, 1).reshape(ROWS, W)
    return out



# revision 4
# speedup vs baseline: 1.3601x; 1.3601x over previous
"""Bilateral blur (7x7, L1 color distance) on 8 Trainium2 NeuronCores.

Input (4, 3, 512, 512) fp32 -> output (4, 3, 512, 512) fp32.
Sharding: pure data parallelism - core i handles batch i//2, row-half i%2
(256 output rows x 512 cols). The host reflect-pads each image and re-lays
each shard into a "transposed-blocked" layout: partition p (0..127) owns
output columns [4p, 4p+4); its free dim holds, per channel, a 10x262 patch
(padded cols 4p..4p+9 x 262 padded rows, flattened xl*262+y), so all 49 tap
shifts (dy, dx) become pure free-dim AP offsets. Two fp16 copies (xe, and
xo shifted one row) keep every hot DVE access-pattern 4-byte aligned, the
DVE fp16 2x-mode requirement.

Key optimizations over the straightforward per-tap pipeline:

1. One-activation Gaussian: erf'(x) = (2/sqrt(pi)) exp(-x^2), so the color
   weight u = exp(-50 d^2) is a single Derivative_Erf (scale=sqrt(50))
   instead of Square+Exp. The space-kernel factor s_k*sqrt(pi)/2 is folded
   into per-tap SCALED IDENTITY matmul stationaries (10 distinct values by
   kernel symmetry).

2. Mirror-symmetry weight reuse: u_k(p) = u_{48-k}(p + d_k), so the whole
   distance pipeline (sub, |.|, channel-sum) + Derivative_Erf runs only for
   the 24 lower-half taps + never for the center, each on an extended
   domain D_k = O u (O - d_k) (4+|dx| cols x 256+|dy| rows ~ 0.73x the
   two-tap work). The mirrored tap reads u_k at AP offset -d_k; for odd-dy
   pairs an ACT Copy re-aligns a 4x256 window of u to an even offset for
   the DVE multiply (Pool/PE readers don't need alignment).

3. The center tap (u == 1) costs nothing: its 8 matmuls stream the input
   windows (and a ones tile for the denominator column) straight from
   SBUF. Every tap's denominator column is likewise read directly from the
   u buffer by the PE - w is never copied into the product tile.

4. PSUM-bank-decoupled accumulation: each of the 8 banks is an independent
   accumulation stream (start/stop are per-bank), so Pool-fed product
   banks are emitted one pair later than DVE-fed ones and Pool latency
   never stalls the PE. The bulk stage (multiplies + matmuls) of each pair
   is emitted one pair behind the distance pipeline (software pipelining
   of the in-order engine queues).

5. Engine balance (cost-model LP): DVE does sub + both chain-adds + 2 mult
   channels (fp16 TT 2x); ACT does |.| + Derivative_Erf + parity copies;
   Pool does the 3rd mult channel (2 channels on 12 of 48 taps); PE does
   the scaled-identity accumulate. Tail reads PSUM directly.
"""
import numpy as np

import concourse.bass as bass
import concourse.bacc as bacc
import concourse.mybir as mybir
from concourse.tile import TileContext
from concourse import bass_utils

C = 3
B, H, W = 4, 512, 512
KX = KY = 7
PAD = 3
SIGMA_COLOR = 0.1
N_CORES = 8

ROWS = 256
WG = 4
NPART = 128
XE, YE = WG + 2 * PAD, ROWS + 2 * PAD
FREE_IN = XE * YE
FREE_OUT = WG * ROWS
NTAP = KY * KX

XWMAX = WG + PAD            # 7
YWPMAX = ROWS + PAD + 1     # 260
EWMAX = XWMAX * YWPMAX      # 1820


def _space_kernel():
    def g1(k, sigma):
        x = np.arange(k, dtype=np.float64) - (k - 1) / 2.0
        g = np.exp(-0.5 * (x / sigma) ** 2)
        return g / g.sum()
    gy, gx = g1(KY, 1.5), g1(KX, 1.5)
    return (gy[:, None] * gx[None, :]).reshape(-1)


def _sidt_table():
    sk = _space_kernel().reshape(KY, KX) * (np.sqrt(np.pi) / 2.0)
    buckets = {}
    k2b = np.zeros(NTAP, np.int32)
    for dy in range(KY):
        for dx in range(KX):
            iy, ix = min(dy, 6 - dy), min(dx, 6 - dx)
            key = (min(iy, ix), max(iy, ix))
            if key not in buckets:
                buckets[key] = (len(buckets), sk[dy, dx])
            k2b[dy * KX + dx] = buckets[key][0]
    vals = np.array([v for _, v in sorted(buckets.values())], np.float64)
    return vals, k2b


_SVALS, _K2B = _sidt_table()
NBKT = len(_SVALS)


def _tap_ap(t, dx, dy, nch=C, ch0=0, dy_base=0):
    a = t[:]
    return bass.AP(a.tensor, a.offset + ch0 * FREE_IN + dx * YE + (dy - dy_base),
                   [[C * FREE_IN, NPART], [FREE_IN, nch], [YE, WG], [1, ROWS]])


def _stk_ap(t, nch=C, ch0=0, step0=False):
    a = t[:]
    tot = a.shape[1]
    return bass.AP(a.tensor, a.offset + ch0 * FREE_OUT,
                   [[tot, NPART], [0 if step0 else FREE_OUT, nch], [ROWS, WG], [1, ROWS]])


def _ext_ap(t, xw, yw, ywp, nch=1, off=0):
    """Ragged AP over an extended-domain buffer: xw col-blocks of stride ywp,
    yw live rows each; optional channel dim of stride EWMAX."""
    a = t[:]
    dims = [[a.shape[1], NPART]]
    if nch > 1:
        dims.append([EWMAX, nch])
    dims += [[ywp, xw], [1, yw]]
    return bass.AP(a.tensor, a.offset + off, dims)


def _build(ntaps=NTAP):
    """ntaps: 49 = full kernel; smaller values emit the center + the first
    (ntaps-1)//2 mirror pairs (used by the delta timer)."""
    nc = bacc.Bacc()
    f32 = mybir.dt.float32
    f16 = mybir.dt.float16
    xe = nc.dram_tensor("xe", [NPART, C * FREE_IN], f16, kind="ExternalInput")
    xo = nc.dram_tensor("xo", [NPART, C * FREE_IN], f16, kind="ExternalInput")
    sidt = nc.dram_tensor("sidt", [NPART, NBKT * NPART], f16, kind="ExternalInput")
    o = nc.dram_tensor("o", [NPART, C * FREE_OUT], f32, kind="ExternalOutput")
    AOT = mybir.AluOpType
    AFT = mybir.ActivationFunctionType
    F = FREE_OUT
    SQ50 = float(np.sqrt(0.5) / SIGMA_COLOR)
    MN = 512
    NPAIR = min(24, max(1, (ntaps - 1) // 2)) if ntaps > 1 else 0
    # tuned placement: 12 tap slots hand a second mult channel to Pool
    add1_pool = set()
    abs_dve = set()
    n_mult_pool2 = 12
    mult_pool2 = {round(i * 48 / n_mult_pool2 + 0.3) % 48
                  for i in range(n_mult_pool2)}

    # pair order: a To-only pair first (its sub can start before the Te DMA
    # lands), then the rest interleaved to spread the odd-dy ACT copies.
    order = [10, 0, 7, 1, 14, 8, 2, 15, 9, 21, 3, 16, 11, 4, 22, 17, 12, 5,
             18, 23, 13, 6, 19, 20][:NPAIR]

    with TileContext(nc) as tc:
        with tc.tile_pool(name="persist", bufs=1) as pool, \
             tc.tile_pool(name="big", bufs=4) as bp, \
             tc.tile_pool(name="sml", bufs=4) as sp, \
             tc.tile_pool(name="mp", bufs=6) as mp, \
             tc.tile_pool(name="ps", bufs=1, space="PSUM") as psp:
            To = pool.tile([NPART, C * FREE_IN], f16, name="To")
            nc.sync.dma_start(To[:], xo[:])
            Te = pool.tile([NPART, C * FREE_IN], f16, name="Te")
            nc.sync.dma_start(Te[:], xe[:])
            sid = pool.tile([NPART, NBKT * NPART], f16, name="sid")
            nc.sync.dma_start(sid[:], sidt[:])
            ones = pool.tile([NPART, MN], f16, name="ones")
            nc.vector.memset(ones[:], 1.0)

            def tile_for(dy):
                return (Te, 0) if dy % 2 == 0 else (To, 1)

            acc = psp.tile([NPART, 4 * F], f32, name="acc")

            def mm(g, bkt, mov, first, last):
                nc.tensor.matmul(acc[:, g * MN:(g + 1) * MN],
                                 sid[:, bkt * NPART:(bkt + 1) * NPART],
                                 mov, start=first, stop=last)

            def accum_tap(bkt, moving_aps, first, last):
                for g in [6, 7, 0, 1, 2, 3, 4, 5]:
                    mm(g, bkt, moving_aps[g], first, last)

            # ---- center tap first (start=True): u == 1, stream inputs ----
            # product banks g=0..5: channel c = g//2, x-cols 2(g%2)..2(g%2)+1
            ctr_movs = []
            for g in range(6):
                c, xh = g // 2, g % 2
                a = To[:]
                ctr_movs.append(bass.AP(
                    a.tensor, a.offset + c * FREE_IN + (2 * xh + PAD) * YE + (PAD - 1),
                    [[C * FREE_IN, NPART], [YE, 2], [1, ROWS]]))
            ctr_movs += [ones[:], ones[:]]
            accum_tap(int(_K2B[24]), ctr_movs, True, ntaps == 1)

            # ---- 24 mirror pairs, bulk stage lagged by one pair ----
            pending = []
            pending_ch2 = []
            for pi in range(NPAIR):
                k = order[pi]
                dy, dx = k // KX, k % KX
                dys, dxs = dy - PAD, dx - PAD
                km = 48 - k
                x0 = min(0, -dxs)
                xw = WG + abs(dxs)
                yw = ROWS + abs(dys)
                ywp = yw + (yw & 1)

                tt, tb = tile_for(dy)
                dlt = bp.tile([NPART, C * EWMAX], f16, name="dlt", tag="dlt")
                nc.vector.tensor_tensor(
                    out=_ext_ap(dlt, xw, yw, ywp, nch=C),
                    in0=bass.AP(tt[:].tensor,
                                tt[:].offset + (x0 + PAD + dxs) * YE + (PAD + dys - tb),
                                [[C * FREE_IN, NPART], [FREE_IN, C], [YE, xw], [1, yw]]),
                    in1=bass.AP(To[:].tensor,
                                To[:].offset + (x0 + PAD) * YE + (PAD - 1),
                                [[C * FREE_IN, NPART], [FREE_IN, C], [YE, xw], [1, yw]]),
                    op=AOT.subtract)
                # |.| in place on dlt (same AP in/out -> per-lane safe)
                adl = dlt
                if pi in abs_dve:
                    nc.vector.tensor_scalar(
                        out=_ext_ap(adl, xw, yw, ywp, nch=C).bitcast(mybir.dt.uint16),
                        in0=_ext_ap(dlt, xw, yw, ywp, nch=C).bitcast(mybir.dt.uint16),
                        scalar1=0x7FFF, scalar2=None, op0=AOT.bitwise_and)
                else:
                    nc.scalar.activation(_ext_ap(adl, xw, yw, ywp, nch=C),
                                         _ext_ap(dlt, xw, yw, ywp, nch=C),
                                         AFT.Abs, bias=0.0, scale=1.0)
                s01 = sp.tile([NPART, EWMAX], f16, name="s01", tag="s01")
                eng1 = nc.gpsimd if pi in add1_pool else nc.vector
                eng1.tensor_tensor(out=_ext_ap(s01, xw, yw, ywp),
                                   in0=_ext_ap(adl, xw, yw, ywp),
                                   in1=_ext_ap(adl, xw, yw, ywp, off=EWMAX),
                                   op=AOT.add)
                dsum = sp.tile([NPART, EWMAX], f16, name="dsum", tag="dsum")
                nc.vector.tensor_tensor(out=_ext_ap(dsum, xw, yw, ywp),
                                        in0=_ext_ap(s01, xw, yw, ywp),
                                        in1=_ext_ap(adl, xw, yw, ywp, off=2 * EWMAX),
                                        op=AOT.add)
                U = sp.tile([NPART, EWMAX], f16, name="U", tag="U")
                nc.scalar.activation(_ext_ap(U, xw, yw, ywp),
                                     _ext_ap(dsum, xw, yw, ywp),
                                     AFT.Derivative_Erf, bias=0.0, scale=SQ50)
                need_sh = abs(dys) % 2 == 1
                mb_x0 = (-dxs - x0) * ywp
                if need_sh:
                    # ACT copies exactly the 4x256 window the DVE mult will
                    # read, re-aligning it to an even base offset. Pool/PE
                    # readers of u don't need alignment and read U directly.
                    Ush = sp.tile([NPART, EWMAX], f16, name="Ush", tag="Ush")
                    nc.scalar.activation(
                        _ext_ap(Ush, WG, ROWS, ywp),
                        _ext_ap(U, WG, ROWS, ywp, off=mb_x0 + abs(dys)),
                        AFT.Copy, bias=0.0, scale=1.0)

                bkt = int(_K2B[k])
                last_pair = pi == NPAIR - 1

                def emit_tap(tdy, tdx, w_t, w_base, wd_t, wd_base, slot,
                             last_other=False, _ywp=ywp, _bkt=bkt):
                    """Emits mults + den/DVE-fed matmuls; returns a closure
                    that emits the Pool-fed bank (ch2) matmuls, to be flushed
                    one pair later so Pool latency never stalls the PE.
                    w_t/w_base: u source for Pool + PE den (alignment-free);
                    wd_t/wd_base: u source for the DVE mult (even offset)."""
                    ta, tba = tile_for(tdy)
                    m = mp.tile([NPART, 3 * F], f16, name="mAll", tag="mAll")
                    wv, wdv = w_t[:], wd_t[:]
                    pool2 = slot in mult_pool2
                    ndve = 1 if pool2 else 2
                    w_bd = bass.AP(wdv.tensor, wdv.offset + wd_base,
                                   [[wdv.shape[1], NPART], [0, ndve], [_ywp, WG], [1, ROWS]])
                    nc.vector.tensor_tensor(
                        out=_stk_ap(m, nch=ndve), in0=w_bd,
                        in1=_tap_ap(ta, tdx, tdy, nch=ndve, dy_base=tba),
                        op=AOT.mult)
                    w_bp = bass.AP(wv.tensor, wv.offset + w_base,
                                   [[wv.shape[1], NPART], [0, 3 - ndve], [_ywp, WG], [1, ROWS]])
                    nc.gpsimd.tensor_tensor(
                        out=_stk_ap(m, nch=3 - ndve, ch0=ndve), in0=w_bp,
                        in1=_tap_ap(ta, tdx, tdy, nch=3 - ndve, ch0=ndve, dy_base=tba),
                        op=AOT.mult)
                    # den banks (u direct) + DVE-fed product banks now
                    for xh in range(2):
                        mm(6 + xh, _bkt,
                           bass.AP(wv.tensor, wv.offset + w_base + 2 * xh * _ywp,
                                   [[wv.shape[1], NPART], [_ywp, 2], [1, ROWS]]),
                           False, last_other)
                    for g in range(4):
                        mm(g, _bkt, m[:, g * MN:(g + 1) * MN], False, last_other)

                    def flush_ch2(last):
                        for g in (4, 5):
                            mm(g, _bkt, m[:, g * MN:(g + 1) * MN], False, last)
                    return flush_ch2

                if need_sh:
                    mir = (6 - dy, 6 - dx, U, mb_x0 + abs(dys), Ush, 0)
                else:
                    mir = (6 - dy, 6 - dx, U, mb_x0 + abs(dys), U, mb_x0 + abs(dys))
                direct = (dy, dx, U, (-x0) * ywp, U, (-x0) * ywp)
                pending.append((emit_tap, direct, mir))
                # flush bulk stage of the previous pair (lag 1); its ch2
                # matmuls flush one pair later still
                if len(pending) > 1:
                    et, d_args, m_args = pending.pop(0)
                    f1 = et(*d_args, 2 * (pi - 1))
                    f2 = et(*m_args, 2 * (pi - 1) + 1)
                    pending_ch2.extend((f1, f2))
                    while len(pending_ch2) > 2:
                        pending_ch2.pop(0)(False)

            # drain: final pair's bulk + all remaining ch2 banks
            if pending:
                et, d_args, m_args = pending.pop(0)
                f1 = et(*d_args, 2 * (NPAIR - 1))
                f2 = et(*m_args, 2 * (NPAIR - 1) + 1, last_other=True)
                pending_ch2.extend((f1, f2))
            while pending_ch2:
                pending_ch2.pop(0)(len(pending_ch2) == 1)

            # tail: recip as soon as the den banks stop; per-channel out,
            # one channel multiplied on Pool to shorten the DVE tail
            recip = pool.tile([NPART, F], f32, name="recip")
            nc.vector.reciprocal(recip[:], acc[:, 3 * F:])
            ot = pool.tile([NPART, C * F], f32, name="ot")
            for c in range(C):
                eng = nc.vector
                eng.tensor_tensor(
                    out=_stk_ap(ot, nch=1, ch0=c),
                    in0=bass.AP(acc[:].tensor, acc[:].offset + c * F,
                                [[4 * F, NPART], [ROWS, WG], [1, ROWS]]),
                    in1=bass.AP(recip[:].tensor, recip[:].offset,
                                [[F, NPART], [ROWS, WG], [1, ROWS]]),
                    op=AOT.mult)
                nc.sync.dma_start(o[:, c * F:(c + 1) * F], ot[:, c * F:(c + 1) * F])
    return nc


_COLIDX = np.arange(NPART)[:, None] * WG + np.arange(XE)[None, :]


def _shard_layout(shard, yshift):
    buf = np.zeros((NPART, C, XE, YE), np.float16)
    for c in range(C):
        blk = shard[c].T[_COLIDX]
        if yshift:
            buf[:, c, :, :YE - yshift] = blk[:, :, yshift:]
        else:
            buf[:, c] = blk
    return buf.reshape(NPART, C * FREE_IN)


def _sidt_payload():
    out = np.zeros((NPART, NBKT * NPART), np.float16)
    for b in range(NBKT):
        out[:, b * NPART:(b + 1) * NPART] = np.eye(NPART) * _SVALS[b]
    return out


_SIDT = _sidt_payload()

_NC_CACHE = {}


def _get_nc():
    if "nc" not in _NC_CACHE:
        nc = _build()
        nc.finalize()
        _NC_CACHE["nc"] = nc
    return _NC_CACHE["nc"]


def make_in_maps(x):
    xp = np.pad(x, ((0, 0), (0, 0), (PAD, PAD), (PAD, PAD)), mode="reflect")
    in_maps = []
    for core in range(N_CORES):
        b, half = core // 2, core % 2
        r0 = half * ROWS
        shard = xp[b, :, r0:r0 + ROWS + 2 * PAD, :]
        in_maps.append({"xe": _shard_layout(shard, 0),
                        "xo": _shard_layout(shard, 1),
                        "sidt": _SIDT})
    return in_maps


def kernel(input: np.ndarray) -> np.ndarray:
    x = np.asarray(input, dtype=np.float32)
    assert x.shape == (B, C, H, W)
    in_maps = make_in_maps(x)
    nc = _get_nc()
    res = bass_utils.run_bass_kernel_spmd(nc, in_maps, list(range(N_CORES)))
    out = np.empty((B, C, H, W), np.float32)
    for core in range(N_CORES):
        b, half = core // 2, core % 2
        r0 = half * ROWS
        ov = np.asarray(res.results[core]["o"]).reshape(NPART, C, WG, ROWS)
        for c in range(C):
            out[b, c, r0:r0 + ROWS, :] = ov[:, c].transpose(2, 0, 1).reshape(ROWS, W)
    return out
